# Initial kernel scaffold
#
"""Grouped-experts SwiGLU FFN (MoE) on 8 Trainium2 NeuronCores.

Expert-parallel: core e owns expert e's weights and its contiguous token
slice.  Tokens are already sorted by expert (contiguous ranges from
cumsum(num_tokens_per_expert)), so the "all-to-all dispatch" is plain host
slicing.  Each core runs a two-stage SwiGLU:

  stage 1:  HT[h, t] = silu(W1 x)[h, t] * (W3 x)[h, t]     (K = DIM)
  stage 2:  OUT.T[d, t] = (W2 @ H)[d, t]                   (K = HIDDEN)

Matmuls run in bf16 (1 cycle/row on the PE array, fp32 PSUM accumulate);
~4e-3 relative error vs the fp32 reference.  Host pre-packs x and weights
into SBUF-tile layout [128, ...] so every DMA reads contiguous lines; the
kernel returns OUT.T per core and the host transposes/scatters back.
"""

import numpy as np
import ml_dtypes

import concourse.bass as bass
from concourse import bacc
import concourse.mybir as mybir
from concourse.tile import TileContext
from concourse.bass_utils import run_bass_kernel_spmd

N_TOKENS = 16384
DIM = 2048
HIDDEN = 1408
N_EXPERTS = 8
N_CORES = 8

P = 128
T = 2048                 # token capacity per core per pass
N_DN = DIM // P          # 16 contraction blocks in stage 1
N_HT = HIDDEN // P       # 11 h tiles
N_DT = DIM // P          # 16 output-row tiles in stage 2
TSUB = 512               # moving-operand width per matmul (1 PSUM bank)

F32 = mybir.dt.float32
BF16 = mybir.dt.bfloat16
SILU = mybir.ActivationFunctionType.Silu
BF = ml_dtypes.bfloat16


def _build_program() -> bass.Bass:
    nc = bacc.Bacc()
    xtp = nc.declare_dram_parameter(
        "xtp", [P, T // TSUB, N_DN, TSUB], BF16, isOutput=False)
    w1p = nc.declare_dram_parameter("w1p", [P, N_HT, N_DN, P], BF16, isOutput=False)
    w3p = nc.declare_dram_parameter("w3p", [P, N_HT, N_DN, P], BF16, isOutput=False)
    w2p = nc.declare_dram_parameter("w2p", [P, N_DT, N_HT, P], BF16, isOutput=False)
    outt = nc.declare_dram_parameter("outt", [DIM, T], F32, isOutput=True)

    with TileContext(nc) as tc:
        with (
            tc.tile_pool(name="xt", bufs=1) as xt_pool,
            tc.tile_pool(name="ht", bufs=1) as ht_pool,
            tc.tile_pool(name="w1", bufs=3) as w1_pool,
            tc.tile_pool(name="w3", bufs=3) as w3_pool,
            tc.tile_pool(name="w2", bufs=3) as w2_pool,
            tc.tile_pool(name="tmp", bufs=3) as tmp_pool,
            tc.tile_pool(name="ob", bufs=3) as ob_pool,
            tc.tile_pool(name="ps1", bufs=2, space="PSUM") as ps1_pool,
            tc.tile_pool(name="ps2", bufs=2, space="PSUM") as ps2_pool,
            tc.tile_pool(name="pso", bufs=2, space="PSUM") as pso_pool,
            tc.tile_pool(name="dmy", bufs=1) as dmy_pool,
            tc.tile_pool(name="psd", bufs=1, space="PSUM") as psd_pool,
        ):
            # PE pre-warm: dep-free dummy matmuls run during the initial DMA
            # wait, releasing the HAM clock throttle (~3.4us busy window)
            # before real operands land
            dmy = dmy_pool.tile([P, TSUB], BF16)
            nc.gpsimd.memset(dmy[:], 0.0)
            for _ in range(7):
                psd = psd_pool.tile([P, TSUB], F32)
                nc.tensor.matmul(psd[:], lhsT=dmy[:, 0:P], rhs=dmy[:])

            # DMA queue order = program order: interleave quarter-loads of
            # (w1, x-chunk0, w3) so the first matmuls' operands land first
            # (~0.8MB), then stream the rest behind PE work
            w1b0 = w1_pool.tile([P, N_DN, P], BF16, tag="w1")
            w3b0 = w3_pool.tile([P, N_DN, P], BF16, tag="w3")
            xcs = [
                xt_pool.tile([P, N_DN, TSUB], BF16, bufs=T // TSUB,
                             tag="xt", name=f"xc{i}")
                for i in range(T // TSUB)
            ]
            for q in range(4):
                dn = slice(q * 4, (q + 1) * 4)
                nc.sync.dma_start(out=w1b0[:, dn, :], in_=w1p[:, 0, dn, :])
                nc.sync.dma_start(out=xcs[0][:, dn, :], in_=xtp[:, 0, dn, :])
                nc.sync.dma_start(out=w3b0[:, dn, :], in_=w3p[:, 0, dn, :])
                if q >= 1:
                    # stream later ts-chunks between the quarter groups so
                    # chunk k lands just before PE finishes ts-block k-1
                    nc.sync.dma_start(out=xcs[q][:], in_=xtp[:, q, :, :])
            xts = xcs
            ht = ht_pool.tile([P, N_HT, T], BF16)

            # stage 1: HT[h, t] = silu(x @ w1.T).T * (x @ w3.T).T
            for ih in range(N_HT):
                if ih == 0:
                    w1b, w3b = w1b0, w3b0
                else:
                    w1b = w1_pool.tile([P, N_DN, P], BF16, tag="w1")
                    nc.sync.dma_start(out=w1b[:], in_=w1p[:, ih, :, :])
                    w3b = w3_pool.tile([P, N_DN, P], BF16, tag="w3")
                    nc.sync.dma_start(out=w3b[:], in_=w3p[:, ih, :, :])
                for its in range(T // TSUB):
                    ts0 = its * TSUB
                    xt_c = xts[its]
                    ps1 = ps1_pool.tile([P, TSUB], F32)
                    ps2 = ps2_pool.tile([P, TSUB], F32)
                    for n in range(N_DN):
                        nc.tensor.matmul(
                            ps1[:],
                            lhsT=w1b[:, n, :],
                            rhs=xt_c[:, n, :],
                            start=(n == 0),
                            stop=(n == N_DN - 1),
                        )
                    for n in range(N_DN):
                        nc.tensor.matmul(
                            ps2[:],
                            lhsT=w3b[:, n, :],
                            rhs=xt_c[:, n, :],
                            start=(n == 0),
                            stop=(n == N_DN - 1),
                        )
                    tmp = tmp_pool.tile([P, TSUB], F32)
                    nc.scalar.activation(tmp[:], ps1[:], SILU)
                    nc.vector.tensor_mul(
                        ht[:, ih, ts0:ts0 + TSUB], tmp[:], ps2[:]
                    )

            # stage 2: OUT.T[d, t] = sum_h W2T[h, d] * HT[h, t]
            for idt in range(N_DT):
                w2b = w2_pool.tile([P, N_HT, P], BF16)
                nc.sync.dma_start(out=w2b[:], in_=w2p[:, idt, :, :])
                for its in range(T // TSUB):
                    ts0 = its * TSUB
                    pso = pso_pool.tile([P, TSUB], F32)
                    for hn in range(N_HT):
                        nc.tensor.matmul(
                            pso[:],
                            lhsT=w2b[:, hn, :],
                            rhs=ht[:, hn, ts0:ts0 + TSUB],
                            start=(hn == 0),
                            stop=(hn == N_HT - 1),
                        )
                    ob = ob_pool.tile([P, TSUB], F32)
                    nc.vector.tensor_copy(ob[:], pso[:])
                    nc.sync.dma_start(
                        out=outt[idt * P:(idt + 1) * P, ts0:ts0 + TSUB],
                        in_=ob[:],
                    )
    nc.compile()
    return nc


_CACHE: dict = {}


def _get_nc() -> bass.Bass:
    if "nc" not in _CACHE:
        _CACHE["nc"] = _build_program()
    return _CACHE["nc"]


def _pack_weights(w1, w2, w3):
    maps = []
    for e in range(N_EXPERTS):
        maps.append({
            "w1p": np.ascontiguousarray(
                w1[e].reshape(N_HT, P, N_DN, P).transpose(3, 0, 2, 1).astype(BF)),
            "w3p": np.ascontiguousarray(
                w3[e].reshape(N_HT, P, N_DN, P).transpose(3, 0, 2, 1).astype(BF)),
            "w2p": np.ascontiguousarray(
                w2[e].reshape(N_DT, P, N_HT, P).transpose(3, 0, 2, 1).astype(BF)),
        })
    return maps


def kernel(x, w1, w2, w3, num_tokens_per_expert, _trace=False):
    x = np.ascontiguousarray(np.asarray(x, dtype=np.float32))
    w1 = np.ascontiguousarray(np.asarray(w1, dtype=np.float32))
    w2 = np.ascontiguousarray(np.asarray(w2, dtype=np.float32))
    w3 = np.ascontiguousarray(np.asarray(w3, dtype=np.float32))
    counts = np.asarray(num_tokens_per_expert, dtype=np.int64)

    cs = np.cumsum(counts)
    starts = np.minimum(np.concatenate([[0], cs[:-1]]), N_TOKENS)
    ends = np.minimum(cs, N_TOKENS)
    lens = np.maximum(ends - starts, 0)

    wmaps = _pack_weights(w1, w2, w3)
    out = np.zeros((N_TOKENS, DIM), np.float32)
    trace_info = []

    n_passes = max(1, int(np.max(np.ceil(lens / T))))
    for k in range(n_passes):
        in_maps = []
        for e in range(N_EXPERTS):
            s = int(starts[e]) + k * T
            xe = np.zeros((T, DIM), np.float32)
            avail = x[s:s + T]
            if avail.shape[0]:
                xe[:avail.shape[0]] = avail
            # [P, n_chunks, N_DN, TSUB]: xtp[p, c, n, t] = x[c*TSUB+t, n*128+p]
            xtp = np.ascontiguousarray(
                xe.T.reshape(N_DN, P, T // TSUB, TSUB)
                .transpose(1, 2, 0, 3).astype(BF))
            in_maps.append({"xtp": xtp, **wmaps[e]})
        res = run_bass_kernel_spmd(
            _get_nc(), in_maps, list(range(N_CORES)), trace=_trace
        )
        if _trace:
            trace_info.append(res)
        for e in range(N_EXPERTS):
            s = int(starts[e]) + k * T
            cnt = min(int(ends[e]) - s, T)
            if cnt > 0:
                out[s:s + cnt] = res.results[e]["outt"].T[:cnt]

    if _trace:
        return out, trace_info
    return out



# revision 1
# speedup vs baseline: 1.1864x; 1.1864x over previous
"""Grouped-experts SwiGLU FFN (MoE) on 8 Trainium2 NeuronCores.

Expert-parallel: core e owns expert e's weights and its contiguous token
slice.  Tokens are already sorted by expert (contiguous ranges from
cumsum(num_tokens_per_expert)), so the "all-to-all dispatch" is plain host
slicing.  Each core runs a two-stage SwiGLU:

  stage 1:  HT[h, t] = silu(W1 x)[h, t] * (W3 x)[h, t]     (K = DIM)
  stage 2:  OUT.T[d, t] = (W2 @ H)[d, t]                   (K = HIDDEN)

Matmuls run in bf16 (1 cycle/row on the PE array, fp32 PSUM accumulate);
~4e-3 relative error vs the fp32 reference.  Host pre-packs x and weights
into SBUF-tile layout [128, ...] so every DMA reads contiguous lines; the
kernel returns OUT.T per core and the host transposes/scatters back.
"""

import numpy as np
import ml_dtypes

import concourse.bass as bass
from concourse import bacc
import concourse.mybir as mybir
from concourse.tile import TileContext
from concourse.bass_utils import run_bass_kernel_spmd

N_TOKENS = 16384
DIM = 2048
HIDDEN = 1408
N_EXPERTS = 8
N_CORES = 8

P = 128
T = 2048                 # token capacity per core per pass
N_DN = DIM // P          # 16 contraction blocks in stage 1
N_HT = HIDDEN // P       # 11 h tiles
N_DT = DIM // P          # 16 output-row tiles in stage 2
TSUB = 512               # moving-operand width per matmul (1 PSUM bank)

F32 = mybir.dt.float32
BF16 = mybir.dt.bfloat16
SILU = mybir.ActivationFunctionType.Silu
BF = ml_dtypes.bfloat16


def _build_program() -> bass.Bass:
    nc = bacc.Bacc()
    xtp = nc.declare_dram_parameter(
        "xtp", [P, T // TSUB, N_DN, TSUB], BF16, isOutput=False)
    w1p = nc.declare_dram_parameter("w1p", [P, N_HT, N_DN, P], BF16, isOutput=False)
    w3p = nc.declare_dram_parameter("w3p", [P, N_HT, N_DN, P], BF16, isOutput=False)
    w2p = nc.declare_dram_parameter("w2p", [P, N_DT, N_HT, P], BF16, isOutput=False)
    outt = nc.declare_dram_parameter("outt", [DIM, T], F32, isOutput=True)

    with TileContext(nc) as tc:
        with (
            tc.tile_pool(name="xt", bufs=1) as xt_pool,
            tc.tile_pool(name="ht", bufs=1) as ht_pool,
            tc.tile_pool(name="w1", bufs=3) as w1_pool,
            tc.tile_pool(name="w3", bufs=3) as w3_pool,
            tc.tile_pool(name="w2", bufs=3) as w2_pool,
            tc.tile_pool(name="tmp", bufs=3) as tmp_pool,
            tc.tile_pool(name="ob", bufs=3) as ob_pool,
            tc.tile_pool(name="ps1", bufs=2, space="PSUM") as ps1_pool,
            tc.tile_pool(name="ps2", bufs=2, space="PSUM") as ps2_pool,
            tc.tile_pool(name="pso", bufs=2, space="PSUM") as pso_pool,
            tc.tile_pool(name="dmy", bufs=1) as dmy_pool,
            tc.tile_pool(name="psd", bufs=1, space="PSUM") as psd_pool,
        ):
            # PE pre-warm: dep-free dummy matmuls run during the initial DMA
            # wait, releasing the HAM clock throttle (~3.4us busy window)
            # before real operands land
            dmy = dmy_pool.tile([P, TSUB], BF16)
            nc.gpsimd.memset(dmy[:], 0.0)
            for _ in range(7):
                psd = psd_pool.tile([P, TSUB], F32)
                nc.tensor.matmul(psd[:], lhsT=dmy[:, 0:P], rhs=dmy[:])

            # DMA queue order = program order: interleave quarter-loads of
            # (w1, x-chunk0, w3) so the first matmuls' operands land first
            # (~0.8MB), then stream the rest behind PE work
            w1b0 = w1_pool.tile([P, N_DN, P], BF16, tag="w1")
            w3b0 = w3_pool.tile([P, N_DN, P], BF16, tag="w3")
            xcs = [
                xt_pool.tile([P, N_DN, TSUB], BF16, bufs=T // TSUB,
                             tag="xt", name=f"xc{i}")
                for i in range(T // TSUB)
            ]
            for q in range(4):
                dn = slice(q * 4, (q + 1) * 4)
                nc.sync.dma_start(out=w1b0[:, dn, :], in_=w1p[:, 0, dn, :])
                nc.sync.dma_start(out=xcs[0][:, dn, :], in_=xtp[:, 0, dn, :])
                nc.sync.dma_start(out=w3b0[:, dn, :], in_=w3p[:, 0, dn, :])
                if q >= 1:
                    # stream later ts-chunks between the quarter groups so
                    # chunk k lands just before PE finishes ts-block k-1
                    nc.sync.dma_start(out=xcs[q][:], in_=xtp[:, q, :, :])
            xts = xcs
            ht = ht_pool.tile([P, N_HT, T], BF16)

            # stage 1: HT[h, t] = silu(x @ w1.T).T * (x @ w3.T).T
            for ih in range(N_HT):
                if ih == 0:
                    w1b, w3b = w1b0, w3b0
                else:
                    w1b = w1_pool.tile([P, N_DN, P], BF16, tag="w1")
                    nc.sync.dma_start(out=w1b[:], in_=w1p[:, ih, :, :])
                    w3b = w3_pool.tile([P, N_DN, P], BF16, tag="w3")
                    nc.sync.dma_start(out=w3b[:], in_=w3p[:, ih, :, :])
                for its in range(T // TSUB):
                    ts0 = its * TSUB
                    xt_c = xts[its]
                    ps1 = ps1_pool.tile([P, TSUB], F32)
                    ps2 = ps2_pool.tile([P, TSUB], F32)
                    for n in range(N_DN):
                        nc.tensor.matmul(
                            ps1[:],
                            lhsT=w1b[:, n, :],
                            rhs=xt_c[:, n, :],
                            start=(n == 0),
                            stop=(n == N_DN - 1),
                        )
                    for n in range(N_DN):
                        nc.tensor.matmul(
                            ps2[:],
                            lhsT=w3b[:, n, :],
                            rhs=xt_c[:, n, :],
                            start=(n == 0),
                            stop=(n == N_DN - 1),
                        )
                    tmp = tmp_pool.tile([P, TSUB], F32)
                    nc.scalar.activation(tmp[:], ps1[:], SILU)
                    nc.vector.tensor_mul(
                        ht[:, ih, ts0:ts0 + TSUB], tmp[:], ps2[:]
                    )

            # stage 2: OUT.T[d, t] = sum_h W2T[h, d] * HT[h, t]
            for idt in range(N_DT):
                w2b = w2_pool.tile([P, N_HT, P], BF16)
                nc.sync.dma_start(out=w2b[:], in_=w2p[:, idt, :, :])
                for its in range(T // TSUB):
                    ts0 = its * TSUB
                    pso = pso_pool.tile([P, TSUB], F32)
                    for hn in range(N_HT):
                        nc.tensor.matmul(
                            pso[:],
                            lhsT=w2b[:, hn, :],
                            rhs=ht[:, hn, ts0:ts0 + TSUB],
                            start=(hn == 0),
                            stop=(hn == N_HT - 1),
                        )
                    ob = ob_pool.tile([P, TSUB], F32)
                    nc.vector.tensor_copy(ob[:], pso[:])
                    nc.sync.dma_start(
                        out=outt[idt * P:(idt + 1) * P, ts0:ts0 + TSUB],
                        in_=ob[:],
                    )
    nc.compile()
    return nc


_CACHE: dict = {}


def _get_nc() -> bass.Bass:
    if "nc" not in _CACHE:
        _CACHE["nc"] = _build_program()
    return _CACHE["nc"]


def _pack_weights(w1, w2, w3):
    maps = []
    for e in range(N_EXPERTS):
        maps.append({
            "w1p": np.ascontiguousarray(
                w1[e].reshape(N_HT, P, N_DN, P).transpose(3, 0, 2, 1).astype(BF)),
            "w3p": np.ascontiguousarray(
                w3[e].reshape(N_HT, P, N_DN, P).transpose(3, 0, 2, 1).astype(BF)),
            "w2p": np.ascontiguousarray(
                w2[e].reshape(N_DT, P, N_HT, P).transpose(3, 0, 2, 1).astype(BF)),
        })
    return maps


def kernel(x, w1, w2, w3, num_tokens_per_expert, _trace=False):
    x = np.ascontiguousarray(np.asarray(x, dtype=np.float32))
    w1 = np.ascontiguousarray(np.asarray(w1, dtype=np.float32))
    w2 = np.ascontiguousarray(np.asarray(w2, dtype=np.float32))
    w3 = np.ascontiguousarray(np.asarray(w3, dtype=np.float32))
    counts = np.asarray(num_tokens_per_expert, dtype=np.int64)

    cs = np.cumsum(counts)
    starts = np.minimum(np.concatenate([[0], cs[:-1]]), N_TOKENS)
    ends = np.minimum(cs, N_TOKENS)
    lens = np.maximum(ends - starts, 0)

    wmaps = _pack_weights(w1, w2, w3)
    out = np.zeros((N_TOKENS, DIM), np.float32)
    trace_info = []

    n_passes = max(1, int(np.max(np.ceil(lens / T))))
    for k in range(n_passes):
        in_maps = []
        for e in range(N_EXPERTS):
            s = int(starts[e]) + k * T
            xe = np.zeros((T, DIM), np.float32)
            avail = x[s:s + T]
            if avail.shape[0]:
                xe[:avail.shape[0]] = avail
            # [P, n_chunks, N_DN, TSUB]: xtp[p, c, n, t] = x[c*TSUB+t, n*128+p]
            xtp = np.ascontiguousarray(
                xe.T.reshape(N_DN, P, T // TSUB, TSUB)
                .transpose(1, 2, 0, 3).astype(BF))
            in_maps.append({"xtp": xtp, **wmaps[e]})
        res = run_bass_kernel_spmd(
            _get_nc(), in_maps, list(range(N_CORES)), trace=_trace
        )
        if _trace:
            trace_info.append(res)
        for e in range(N_EXPERTS):
            s = int(starts[e]) + k * T
            cnt = min(int(ends[e]) - s, T)
            if cnt > 0:
                out[s:s + cnt] = res.results[e]["outt"].T[:cnt]

    if _trace:
        return out, trace_info
    return out

